# revision 1
# baseline (speedup 1.0000x reference)
"""Causal self-attention (B=4, T=2048, D=1024, H=16) on 8 trn2 NeuronCores.

Sharding: 2 cores per batch element; each core handles 8 heads
(tensor-parallel head split). Each core computes QKV projections for its
heads, causal flash-style attention, and a partial o_proj over its 512
head-dims. Host sums the two partial o_proj outputs per batch element.

Per-core kernel (all matmuls fp32r = full-rate single-pass fp32):
  phase 1: qT/kT (transposed, [e,t]) and v (natural, [t,e']) projections
  phase 2: per head-pair, per 512-col t-block: K=64 row-packed score MMs
           (S^T layout), causal mask via tri-mask add on the diagonal
           block, exp on ACT (scale=1/8, no max subtraction -- scores are
           ~N(0,1) after scale), PV matmul with an appended ones column
           that accumulates the softmax denominator in psum row 64.
  phase 3: batched reciprocal of denominators, DRAM-roundtrip partition
           broadcast, normalize O^T, partial o_proj (+bias on core 0 of
           each pair), DMA out.
"""
import numpy as np

import concourse.bass as bass
import concourse.tile as tile
from concourse import bacc, mybir
from concourse.bass_utils import run_bass_kernel_spmd

F32 = mybir.dt.float32
F32R = mybir.dt.float32r

T = 2048          # sequence length
D = 1024          # d_model
HL = 8            # local heads per core
DK = 64           # head dim
NPAIR = 4         # head pairs per core
NTJ = 4           # t blocks of 512
NSI = 16          # s chunks of 128
NDC = 8           # d_model chunks of 128
NSLAB = 4         # x^T slabs of 512 t-columns

_CACHE: dict = {}


def _r(ap):
    return ap.bitcast(F32R)


def _emit(nc, tc, ctx, ins, outs):
    xT, wqkvT, qkb, vb, woT, bo = ins
    (y,) = outs

    persist = ctx.enter_context(tc.tile_pool(name="persist", bufs=1))
    psA = ctx.enter_context(tc.tile_pool(name="psA", bufs=2, space="PSUM"))
    psPV = ctx.enter_context(tc.tile_pool(name="psPV", bufs=2, space="PSUM"))
    psS = ctx.enter_context(tc.tile_pool(name="psS", bufs=2, space="PSUM"))

    # ---- persistent SBUF regions ----
    qT = persist.tile([128, NPAIR, T], F32, tag="qT")     # [dk-pair, pair, t]
    kT = persist.tile([128, NPAIR, T], F32, tag="kT")
    v_aug = persist.tile([128, NSI, HL, DK + 1], F32, tag="vaug")  # [s, si, h, dk+1]
    tri = persist.tile([128, 128], F32, tag="tri")
    zeros = persist.tile([128, 384], F32, tag="zeros")
    qkb_t = persist.tile([128, 8], F32, tag="qkb")
    vb_t = persist.tile([128, 512], F32, tag="vb")
    # denominators: row = pair*32 + h01*4 + tj (32-aligned per pair)
    stag = persist.tile([128, 512], F32, tag="stag")
    rstag = persist.tile([128, 512], F32, tag="rstag")
    nc.gpsimd.memset(stag[:], 1.0)

    # first x slab load goes ahead of everything else on the gpsimd queue
    from contextlib import ExitStack as _ES
    xs_es = _ES()
    xs_pool = xs_es.enter_context(tc.tile_pool(name="xs", bufs=2))
    xT_r = xT.rearrange("(c p) t -> p c t", p=128)
    xs0 = xs_pool.tile([128, NDC, 512], F32, tag="xs")
    nc.gpsimd.dma_start(out=_r(xs0[:]), in_=_r(xT_r[:, :, 0:512]))

    # masks / constants
    nc.gpsimd.memset(tri[:], 0.0)
    nc.gpsimd.affine_select(
        out=tri[:], in_=tri[:], compare_op=mybir.AluOpType.is_ge,
        fill=-1e30, base=0, pattern=[[1, 128]], channel_multiplier=-1,
    )
    nc.gpsimd.memset(zeros[:], 0.0)
    nc.sync.dma_start(out=qkb_t[:], in_=qkb[:])
    # vb broadcast across partitions (DRAM src, stride-0 partition dim)
    vb_src = bass.AP(tensor=vb.tensor, offset=vb.offset, ap=[[0, 128]] + list(vb.ap))
    nc.gpsimd.dma_start(out=vb_t[:], in_=vb_src)
    # ones column of v_aug ([128, NSI, HL]) via DVE strided copy
    ones_t = persist.tile([128, 128], F32, tag="ones")
    nc.gpsimd.memset(ones_t[:], 1.0)
    nc.vector.tensor_copy(
        _r(v_aug[:, :, :, 64]),
        ones_t[:].rearrange("p (a b) -> p a b", a=NSI),
    )

    # ---- phase 1: projections ----
    with tc.tile_pool(name="ph1", bufs=1) as ph1:
        w_t = ph1.tile([128, NDC, 3 * 512], F32, tag="w")
        # split the weight load so the first projections start sooner;
        # first chunk is just the q/k columns of pair 0
        w_src = wqkvT.rearrange("(c p) e -> p c e", p=128)
        w_chunks = [(0, 128), (512, 640), (128, 512), (640, 1024), (1024, 1536)]
        for lo, hi in w_chunks:
            nc.sync.dma_start(
                out=_r(w_t[:, :, lo:hi]),
                in_=_r(w_src[:, :, lo:hi]),
            )
        def project_qk(xs, ec, t0):
            ps = psA.tile([128, 512], F32, tag="acc")
            for dc in range(NDC):
                nc.tensor.matmul(
                    ps[:], _r(w_t[:, dc, ec * 128:(ec + 1) * 128]),
                    _r(xs[:, dc, :]),
                    start=(dc == 0), stop=(dc == NDC - 1),
                )
            dst = qT if ec < 4 else kT
            pair = ec % 4
            nc.vector.tensor_add(
                _r(dst[:, pair, t0:t0 + 512]), ps[:],
                qkb_t[:, ec:ec + 1].broadcast_to([128, 512]),
            )

        def project_v(xs, tsub, si):
            ps = psA.tile([128, 512], F32, tag="acc")
            for dc in range(NDC):
                nc.tensor.matmul(
                    ps[:], _r(xs[:, dc, tsub * 128:(tsub + 1) * 128]),
                    _r(w_t[:, dc, 1024:1536]),
                    start=(dc == 0), stop=(dc == NDC - 1),
                )
            nc.vector.tensor_add(
                _r(v_aug[:, si, :, 0:64]),
                ps[:].rearrange("p (h c) -> p h c", h=HL),
                vb_t[:].rearrange("p (h c) -> p h c", h=HL),
            )

        def load_slab(slab):
            t0 = slab * 512
            if slab == 0:
                return xs0, t0
            xs = xs_pool.tile([128, NDC, 512], F32, tag="xs")
            nc.gpsimd.dma_start(
                out=_r(xs[:]), in_=_r(xT_r[:, :, t0:t0 + 512]),
            )
            return xs, t0

        for slab in range(NSLAB):
            xs, t0 = load_slab(slab)
            for ec in (0, 4, 1, 5, 2, 6, 3, 7):
                project_qk(xs, ec, t0)
            for tsub in range(4):
                project_v(xs, tsub, slab * 4 + tsub)

    xs_es.close()

    # ---- phase 2: attention (+ per-pair normalization) ----
    ph23 = ctx.enter_context(tc.tile_pool(name="ph23", bufs=1))
    ot = ph23.tile([128, NPAIR, T], F32, tag="ot")  # [d'pair, pair, t]
    # o_proj weights prefetch during attention
    wo_t = ph23.tile([128, NPAIR, D], F32, tag="wo")
    nc.scalar.dma_start(
        out=_r(wo_t[:]), in_=_r(woT.rearrange("(c p) e -> p c e", p=128)),
    )
    bo_t = ph23.tile([128, D], F32, tag="bo")
    bo_src = bass.AP(tensor=bo.tensor, offset=bo.offset,
                     ap=[[0, 128]] + list(bo.ap))
    nc.gpsimd.dma_start(out=bo_t[:], in_=bo_src)
    rec_d = nc.dram_tensor("rec_scratch", [128, 512], F32).ap()
    rec_flat = rec_d.rearrange("a b -> (a b)")
    bc_pool = ctx.enter_context(tc.tile_pool(name="bcpool", bufs=1))
    with tc.tile_pool(name="epool", bufs=3) as e_pool, \
         tc.tile_pool(name="scpool", bufs=2) as sc_pool:
        for pair in range(NPAIR):
            hA, hB = 2 * pair, 2 * pair + 1
            scA = sc_pool.tile([65, NTJ, 512], F32, tag="sc")
            scB = sc_pool.tile([65, NTJ, 512], F32, tag="sc")
            for tj in range(NTJ):
                pvA = psPV.tile([65, 512], F32, tag="pv")
                pvB = psPV.tile([65, 512], F32, tag="pv")
                n_si = 4 * tj + 4
                for si in range(n_si):
                    r = si - 4 * tj  # >=0 on the diagonal block
                    off = 128 * r if r >= 0 else 0
                    n = 512 - off
                    S = psS.tile([128, 2, 512], F32, tag="sco")
                    E = e_pool.tile([128, 2, 512], F32, tag="E")
                    # scores S^T[s, t] for both heads (row-packed K=64)
                    nc.tensor.matmul(
                        S[:, 0, 0:n],
                        _r(kT[0:64, pair, si * 128:(si + 1) * 128]),
                        _r(qT[0:64, pair, tj * 512 + off: (tj + 1) * 512]),
                        start=True, stop=True,
                    )
                    nc.tensor.matmul(
                        S[:, 1, 0:n],
                        _r(kT[64:128, pair, si * 128:(si + 1) * 128]),
                        _r(qT[64:128, pair, tj * 512 + off: (tj + 1) * 512]),
                        start=True, stop=True,
                        tile_position=(64, 0),
                    )
                    if r >= 0:
                        # causal mask on the diagonal 128x128 block
                        nc.vector.tensor_add(S[:, 0, 0:128], S[:, 0, 0:128], tri[:])
                        nc.vector.tensor_add(S[:, 1, 0:128], S[:, 1, 0:128], tri[:])
                    nc.scalar.activation(
                        out=_r(E[:, :, off:512]), in_=S[:, :, 0:n],
                        func=mybir.ActivationFunctionType.Exp, scale=0.125,
                    )
                    if off > 0:
                        # zero the unwritten prefix so PV can run full-width
                        nc.vector.tensor_copy(_r(E[:, 0, 0:off]), zeros[:, 0:off])
                        nc.vector.tensor_copy(_r(E[:, 1, 0:off]), zeros[:, 0:off])
                    nc.tensor.matmul(
                        pvA[:], _r(v_aug[:, si, hA, :]), _r(E[:, 0, :]),
                        start=(si == 0), stop=(si == n_si - 1),
                        skip_group_check=True,
                    )
                    nc.tensor.matmul(
                        pvB[:], _r(v_aug[:, si, hB, :]), _r(E[:, 1, :]),
                        start=(si == 0), stop=(si == n_si - 1),
                        skip_group_check=True,
                    )
                nc.vector.tensor_copy(_r(scA[:, tj, :]), pvA[:])
                nc.vector.tensor_copy(_r(scB[:, tj, :]), pvB[:])
            # batched partition-shift + denominator DMAs (one each per head)
            r0p = pair * 32
            for h01, sc in ((0, scA), (1, scB)):
                nc.sync.dma_start(
                    out=_r(ot[h01 * 64:(h01 + 1) * 64, pair, :]),
                    in_=_r(sc[0:64, :, :].rearrange("p a b -> p (a b)")),
                )
                r0 = r0p + h01 * 4
                nc.scalar.dma_start(out=stag[r0:r0 + 4, :], in_=sc[64:65, :, :])
            # per-pair normalization chain (overlaps later pairs' attention)
            nc.vector.reciprocal(rstag[r0p:r0p + 32, :], stag[r0p:r0p + 32, :])
            nc.sync.dma_start(out=rec_d[r0p:r0p + 32, :],
                              in_=rstag[r0p:r0p + 32, :])
            bct = bc_pool.tile([128, NTJ, 512], F32, tag="bc")
            for h01 in range(2):
                r0 = r0p + h01 * 4
                src = rec_flat[r0 * 512:(r0 + 4) * 512]
                bsrc = bass.AP(tensor=src.tensor, offset=src.offset,
                               ap=[[0, 64]] + list(src.ap))
                nc.gpsimd.dma_start(
                    out=bct[h01 * 64:(h01 + 1) * 64, :, :]
                        .rearrange("p a b -> p (a b)"),
                    in_=bsrc,
                )
            nc.vector.tensor_mul(
                _r(ot[:, pair, :]), ot[:, pair, :],
                bct[:].rearrange("p a b -> p (a b)"),
            )

    # ---- phase 3: o_proj ----
    with tc.tile_pool(name="ypool", bufs=3) as y_pool:
        for tc_ in range(16):
            ys = y_pool.tile([128, 1024], F32, tag="ys")
            for ec in range(2):
                ps = psA.tile([128, 512], F32, tag="acc")
                for pair in range(NPAIR):
                    nc.tensor.matmul(
                        ps[:], _r(ot[:, pair, tc_ * 128:(tc_ + 1) * 128]),
                        _r(wo_t[:, pair, ec * 512:(ec + 1) * 512]),
                        start=(pair == 0), stop=(pair == NPAIR - 1),
                    )
                nc.vector.tensor_add(
                    ys[:, ec * 512:(ec + 1) * 512], ps[:],
                    bo_t[:, ec * 512:(ec + 1) * 512],
                )
            eng = nc.sync if tc_ % 2 == 0 else nc.scalar
            eng.dma_start(
                out=y[tc_ * 128:(tc_ + 1) * 128, :], in_=ys[:],
            )


def _build():
    if "nc" in _CACHE:
        return _CACHE["nc"]
    from contextlib import ExitStack

    nc = bacc.Bacc("TRN2", target_bir_lowering=False, debug=False, num_devices=8)
    xT = nc.dram_tensor("xT", [D, T], F32, kind="ExternalInput").ap()
    wqkvT = nc.dram_tensor("wqkvT", [D, 3 * 512], F32, kind="ExternalInput").ap()
    qkb = nc.dram_tensor("qkb", [128, 8], F32, kind="ExternalInput").ap()
    vb = nc.dram_tensor("vb", [512], F32, kind="ExternalInput").ap()
    woT = nc.dram_tensor("woT", [512, D], F32, kind="ExternalInput").ap()
    bo = nc.dram_tensor("bo", [D], F32, kind="ExternalInput").ap()
    y = nc.dram_tensor("y", [T, D], F32, kind="ExternalOutput").ap()

    with tile.TileContext(nc) as tc:
        with ExitStack() as ctx:
            _emit(nc, tc, ctx, (xT, wqkvT, qkb, vb, woT, bo), (y,))
    nc.compile()
    _CACHE["nc"] = nc
    return nc


def _shard_inputs(x, Wqkv, bqkv, Wo, bo):
    """Build the 8 per-core input maps."""
    x = np.ascontiguousarray(np.asarray(x, dtype=np.float32))
    Wqkv = np.asarray(Wqkv, dtype=np.float32)
    bqkv = np.asarray(bqkv, dtype=np.float32)
    Wo = np.asarray(Wo, dtype=np.float32)
    bo = np.asarray(bo, dtype=np.float32)

    in_maps = []
    for core in range(8):
        b, hg = core // 2, core % 2
        heads = hg * 8 + np.arange(8)
        rows = (heads[:, None] * 64 + np.arange(64)[None, :]).ravel()  # 512
        q_rows, k_rows, v_rows = rows, 1024 + rows, 2048 + rows
        in_maps.append({
            "xT": np.ascontiguousarray(x[b].T),
            "wqkvT": np.ascontiguousarray(
                Wqkv[np.concatenate([q_rows, k_rows, v_rows])].T),
            "qkb": np.ascontiguousarray(
                bqkv[np.concatenate([q_rows, k_rows])].reshape(8, 128).T),
            "vb": np.ascontiguousarray(bqkv[v_rows]),
            "woT": np.ascontiguousarray(Wo[:, rows].T),
            "bo": (bo if hg == 0 else np.zeros_like(bo)),
        })
    return in_maps


def _get_runner():
    """Build (once) a cached jitted 8-core runner mirroring
    bass2jax.run_bass_via_pjrt, so repeat calls skip retracing."""
    if "runner" in _CACHE:
        return _CACHE["runner"]
    import jax
    from jax.sharding import Mesh, PartitionSpec
    from jax.experimental.shard_map import shard_map
    from concourse import bass2jax as b2j
    from concourse import mybir as _mb

    nc = _build()
    b2j.install_neuronx_cc_hook()
    partition_name = nc.partition_id_tensor.name if nc.partition_id_tensor else None

    in_names, out_names, out_avals, zero_shapes = [], [], [], []
    for alloc in nc.m.functions[0].allocations:
        if not isinstance(alloc, _mb.MemoryLocationSet):
            continue
        name = alloc.memorylocations[0].name
        if alloc.kind == "ExternalInput":
            if name != partition_name:
                in_names.append(name)
        elif alloc.kind == "ExternalOutput":
            shape = tuple(alloc.tensor_shape)
            dtype = _mb.dt.np(alloc.dtype)
            out_names.append(name)
            out_avals.append(jax.core.ShapedArray(shape, dtype))
            zero_shapes.append((shape, dtype))
    n_params = len(in_names)
    all_names = list(in_names) + list(out_names)
    if partition_name is not None:
        all_names.append(partition_name)

    def _body(*args):
        operands = list(args)
        if partition_name is not None:
            operands.append(b2j.partition_id_tensor())
        outs = b2j._bass_exec_p.bind(
            *operands,
            out_avals=tuple(out_avals),
            in_names=tuple(all_names),
            out_names=tuple(out_names),
            lowering_input_output_aliases=(),
            sim_require_finite=True,
            sim_require_nnan=True,
            nc=nc,
        )
        return tuple(outs)

    devices = jax.devices()[:8]
    mesh = Mesh(np.asarray(devices), ("core",))
    n_outs = len(out_names)
    sharded = jax.jit(
        shard_map(
            _body, mesh=mesh,
            in_specs=(PartitionSpec("core"),) * (n_params + n_outs),
            out_specs=(PartitionSpec("core"),) * n_outs,
            check_rep=False,
        ),
        donate_argnums=tuple(range(n_params, n_params + n_outs)),
        keep_unused=True,
    )
    runner = {
        "sharded": sharded,
        "in_names": in_names,
        "out_names": out_names,
        "zero_shapes": zero_shapes,
        "out_avals": out_avals,
    }
    _CACHE["runner"] = runner
    return runner


def _concat_inputs(in_maps, runner):
    return [
        np.concatenate([in_maps[c][name] for c in range(8)], axis=0)
        for name in runner["in_names"]
    ]


def _fresh_zeros(runner):
    return [np.zeros((8 * s[0], *s[1:]), d) for (s, d) in runner["zero_shapes"]]


def kernel(x, Wqkv, bqkv, Wo, bo):
    runner = _get_runner()
    in_maps = _shard_inputs(x, Wqkv, bqkv, Wo, bo)
    out_arrs = runner["sharded"](*_concat_inputs(in_maps, runner),
                                 *_fresh_zeros(runner))
    yi = runner["out_names"].index("y")
    parts = np.asarray(out_arrs[yi]).reshape(8, T, D)
    out = np.empty((4, T, D), dtype=np.float32)
    for b in range(4):
        out[b] = parts[2 * b] + parts[2 * b + 1]
    return out



# revision 23
# speedup vs baseline: 185.8403x; 185.8403x over previous
"""Causal self-attention (B=4, T=2048, D=1024, H=16) on 8 trn2 NeuronCores.

Sharding: 2 cores per batch element; each core handles 8 heads
(tensor-parallel head split). Each core computes QKV projections for its
heads, causal flash-style attention, and a partial o_proj over its 512
head-dims. Host sums the two partial o_proj outputs per batch element.

All matmul operands (x, Wqkv, Wo, q/k/v, attention weights, o) are bf16;
every accumulation happens in fp32 PSUM; biases and the softmax
normalization stay fp32.

Per-core kernel:
  phase 1: qT/kT (transposed, [e,t]) and v (natural, [t,e']) projections
  phase 2: per head-pair, per 512-col t-block: K=64 row-packed score MMs
           (S^T layout), causal mask via tri-mask add on the diagonal
           block, exp on ACT (scale=1/8, no max subtraction -- scores are
           ~N(0,1) after scale), PV matmul with an appended ones column
           that accumulates the softmax denominator in psum row 64.
  phase 3: batched reciprocal of denominators, DRAM-roundtrip partition
           broadcast, normalize O^T, partial o_proj (+bias on core 0 of
           each pair), DMA out.
"""
import numpy as np

import concourse.bass as bass
import concourse.tile as tile
from concourse import bacc, mybir
from concourse.bass_utils import run_bass_kernel_spmd

F32 = mybir.dt.float32
BF16 = mybir.dt.bfloat16

T = 2048          # sequence length
D = 1024          # d_model
HL = 8            # local heads per core
DK = 64           # head dim
NPAIR = 4         # head pairs per core
NTJ = 4           # t blocks of 512
NSI = 16          # s chunks of 128
NDC = 8           # d_model chunks of 128
NSLAB = 4         # x^T slabs of 512 t-columns

_CACHE: dict = {}


def _emit(nc, tc, ctx, ins, outs, uid=0):
    xT, wqkvT, qkb, vb, woT, bo = ins
    (y,) = outs

    persist = ctx.enter_context(tc.tile_pool(name="persist", bufs=1))

    # ---- persistent SBUF regions ----
    qT = persist.tile([128, NPAIR, T], BF16, tag="qT")     # [dk-pair, pair, t]
    kT = persist.tile([128, NPAIR, T], BF16, tag="kT")
    v_aug = persist.tile([128, NSI, HL, DK + 1], BF16, tag="vaug")  # [s, si, h, dk+1]
    tri = persist.tile([128, 128], F32, tag="tri")
    qkb_t = persist.tile([128, 8], F32, tag="qkb")
    vb_t = persist.tile([128, 512], F32, tag="vb")
    # denominators: row = pair*32 + h01*4 + tj (32-aligned per pair)
    stag = persist.tile([128, 512], BF16, tag="stag")
    rstag = persist.tile([128, 512], F32, tag="rstag")
    nc.gpsimd.memset(stag[:], 1.0)

    # first x slab load goes ahead of everything else on the gpsimd queue
    from contextlib import ExitStack as _ES
    xs_es = _ES()
    xs_pool = xs_es.enter_context(tc.tile_pool(name="xs", bufs=4))
    xT_r = xT.rearrange("(c p) t -> p c t", p=128)
    xs0 = xs_pool.tile([128, NDC, 512], BF16, tag="xs")
    # per-dc chunk DMAs so the dc=0 matmul starts after the first chunk
    for dc in range(NDC):
        nc.gpsimd.dma_start(out=xs0[:, dc, :], in_=xT_r[:, dc, 0:512])

    # masks / constants
    nc.gpsimd.memset(tri[:], 0.0)
    nc.gpsimd.affine_select(
        out=tri[:], in_=tri[:], compare_op=mybir.AluOpType.is_ge,
        fill=-1e30, base=0, pattern=[[1, 128]], channel_multiplier=-1,
    )
    nc.sync.dma_start(out=qkb_t[:], in_=qkb[:])
    # vb broadcast across partitions (DRAM src, stride-0 partition dim)
    vb_src = bass.AP(tensor=vb.tensor, offset=vb.offset, ap=[[0, 128]] + list(vb.ap))
    nc.gpsimd.dma_start(out=vb_t[:], in_=vb_src)
    # ones column of v_aug ([128, NSI, HL]) via DVE strided copy
    ones_t = persist.tile([128, 128], F32, tag="ones")
    nc.gpsimd.memset(ones_t[:], 1.0)
    nc.vector.tensor_copy(
        v_aug[:, :, :, 64],
        ones_t[:].rearrange("p (a b) -> p a b", a=NSI),
    )

    # ---- phase 1: projections ----
    # qk loops are slab-inner so each Wqkv stationary [128,128] is reused
    # across 4 moving slabs before switching (amortizes weight loads)
    with tc.tile_pool(name="ph1", bufs=1) as ph1, \
         tc.tile_pool(name="ps1", bufs=4, space="PSUM") as ps1:
        w_t = ph1.tile([128, NDC, 3 * 512], BF16, tag="w")
        # split the weight load so the first projections start sooner;
        # first chunk is just the q/k columns of pair 0
        w_src = wqkvT.rearrange("(c p) e -> p c e", p=128)
        w_chunks = [(0, 128), (512, 640), (128, 512), (640, 1024), (1024, 1536)]
        for lo, hi in w_chunks:
            nc.sync.dma_start(
                out=w_t[:, :, lo:hi],
                in_=w_src[:, :, lo:hi],
            )

        def load_slab(slab):
            if slab == 0:
                return xs0
            xs = xs_pool.tile([128, NDC, 512], BF16, tag="xs")
            for dc in range(NDC):
                nc.gpsimd.dma_start(
                    out=xs[:, dc, :], in_=xT_r[:, dc, slab * 512:(slab + 1) * 512],
                )
            return xs

        xs_all = [load_slab(s) for s in range(NSLAB)]

        for ec in (0, 4, 1, 5, 2, 6, 3, 7):
            groups = [ps1.tile([128, 512], F32, tag="acc", name=f"acc_{ec}_{s}")
                      for s in range(NSLAB)]
            for dc in range(NDC):
                for slab in range(NSLAB):
                    nc.tensor.matmul(
                        groups[slab][:], w_t[:, dc, ec * 128:(ec + 1) * 128],
                        xs_all[slab][:, dc, :],
                        start=(dc == 0), stop=(dc == NDC - 1),
                    )
            dst = qT if ec < 4 else kT
            pair = ec % 4
            for slab in range(NSLAB):
                nc.vector.tensor_add(
                    dst[:, pair, slab * 512:slab * 512 + 512], groups[slab][:],
                    qkb_t[:, ec:ec + 1].broadcast_to([128, 512]),
                )

        def project_v(xs, tsub, si):
            ps = ps1.tile([128, 512], F32, tag="vacc")
            for dc in range(NDC):
                nc.tensor.matmul(
                    ps[:], xs[:, dc, tsub * 128:(tsub + 1) * 128],
                    w_t[:, dc, 1024:1536],
                    start=(dc == 0), stop=(dc == NDC - 1),
                )
            nc.vector.tensor_add(
                v_aug[:, si, :, 0:64],
                ps[:].rearrange("p (h c) -> p h c", h=HL),
                vb_t[:].rearrange("p (h c) -> p h c", h=HL),
            )

        for slab in range(NSLAB):
            for tsub in range(4):
                project_v(xs_all[slab], tsub, slab * 4 + tsub)

    xs_es.close()

    # ---- phase 2: attention (+ per-pair normalization) ----
    ph23 = ctx.enter_context(tc.tile_pool(name="ph23", bufs=1))
    ot = ph23.tile([128, NPAIR, T], BF16, tag="ot")  # [d'pair, pair, t]
    # o_proj weights prefetch during attention
    wo_t = ph23.tile([128, NPAIR, D], BF16, tag="wo")
    nc.scalar.dma_start(
        out=wo_t[:], in_=woT.rearrange("(c p) e -> p c e", p=128),
    )
    bo_t = ph23.tile([128, D], F32, tag="bo")
    bo_src = bass.AP(tensor=bo.tensor, offset=bo.offset,
                     ap=[[0, 128]] + list(bo.ap))
    nc.gpsimd.dma_start(out=bo_t[:], in_=bo_src)
    rec_d = nc.dram_tensor(f"rec_scratch_{uid}", [128, 512], F32).ap()
    rec_flat = rec_d.rearrange("a b -> (a b)")
    bc_pool = ctx.enter_context(tc.tile_pool(name="bcpool", bufs=1))
    with tc.tile_pool(name="epool", bufs=3) as e_pool, \
         tc.tile_pool(name="scpool", bufs=3) as sc_pool, \
         tc.tile_pool(name="psPV", bufs=2, space="PSUM") as psPV, \
         tc.tile_pool(name="psS", bufs=2, space="PSUM") as psS:
        for pair in range(NPAIR):
            hA, hB = 2 * pair, 2 * pair + 1
            scA = sc_pool.tile([65, NTJ, 512], BF16, tag="sc")
            scB = sc_pool.tile([65, NTJ, 512], BF16, tag="sc")
            for tj in range(NTJ):
                pvA = psPV.tile([65, 512], F32, tag="pv")
                pvB = psPV.tile([65, 512], F32, tag="pv")
                n_si = 4 * tj + 4

                def emit_scores(si):
                    """Scores + mask + exp for one s-chunk; returns (E, off)."""
                    r = si - 4 * tj  # >=0 on the diagonal block
                    off = 128 * r if r >= 0 else 0
                    n = 512 - off
                    S = psS.tile([128, 2, 512], F32, tag="sco")
                    E = e_pool.tile([128, 2, 512], BF16, tag="E")
                    # scores S^T[s, t] for both heads (row-packed K=64)
                    nc.tensor.matmul(
                        S[:, 0, 0:n],
                        kT[0:64, pair, si * 128:(si + 1) * 128],
                        qT[0:64, pair, tj * 512 + off: (tj + 1) * 512],
                        start=True, stop=True,
                    )
                    nc.tensor.matmul(
                        S[:, 1, 0:n],
                        kT[64:128, pair, si * 128:(si + 1) * 128],
                        qT[64:128, pair, tj * 512 + off: (tj + 1) * 512],
                        start=True, stop=True,
                        tile_position=(64, 0),
                    )
                    if r >= 0:
                        # causal mask on the diagonal 128x128 block
                        nc.vector.tensor_add(S[:, 0, 0:128], S[:, 0, 0:128], tri[:])
                        nc.vector.tensor_add(S[:, 1, 0:128], S[:, 1, 0:128], tri[:])
                    nc.scalar.activation(
                        out=E[:, :, off:512], in_=S[:, :, 0:n],
                        func=mybir.ActivationFunctionType.Exp, scale=0.125,
                    )
                    return E, off

                def emit_pv(si, E, off):
                    # PV streams only the valid suffix; columns [0:off] of the
                    # diagonal blocks are causally masked (exp -> 0) so they
                    # contribute nothing and are skipped entirely
                    nc.tensor.matmul(
                        pvA[:, off:512], v_aug[:, si, hA, :],
                        E[:, 0, off:512],
                        start=(si == 0), stop=(si == n_si - 1),
                        skip_group_check=True,
                    )
                    nc.tensor.matmul(
                        pvB[:, off:512], v_aug[:, si, hB, :],
                        E[:, 1, off:512],
                        start=(si == 0), stop=(si == n_si - 1),
                        skip_group_check=True,
                    )

                # software pipeline: scores of si+1 are emitted before the
                # PV of si, so the in-order PE queue streams the next score
                # block while ACT computes the current exp
                prev = None
                for si in range(n_si):
                    cur = (si, *emit_scores(si))
                    if prev is not None:
                        emit_pv(*prev)
                    prev = cur
                emit_pv(*prev)
                nc.vector.tensor_copy(scA[:, tj, :], pvA[:])
                nc.vector.tensor_copy(scB[:, tj, :], pvB[:])
            # batched partition-shift + denominator DMAs (one each per head)
            r0p = pair * 32
            for h01, sc in ((0, scA), (1, scB)):
                nc.sync.dma_start(
                    out=ot[h01 * 64:(h01 + 1) * 64, pair, :],
                    in_=sc[0:64, :, :].rearrange("p a b -> p (a b)"),
                )
                r0 = r0p + h01 * 4
                nc.scalar.dma_start(out=stag[r0:r0 + 4, :], in_=sc[64:65, :, :])
            # per-pair normalization chain (overlaps later pairs' attention)
            nc.vector.reciprocal(rstag[r0p:r0p + 32, :], stag[r0p:r0p + 32, :])
            nc.sync.dma_start(out=rec_d[r0p:r0p + 32, :],
                              in_=rstag[r0p:r0p + 32, :])
            bct = bc_pool.tile([128, NTJ, 512], F32, tag="bc")
            for h01 in range(2):
                r0 = r0p + h01 * 4
                src = rec_flat[r0 * 512:(r0 + 4) * 512]
                bsrc = bass.AP(tensor=src.tensor, offset=src.offset,
                               ap=[[0, 64]] + list(src.ap))
                nc.gpsimd.dma_start(
                    out=bct[h01 * 64:(h01 + 1) * 64, :, :]
                        .rearrange("p a b -> p (a b)"),
                    in_=bsrc,
                )
            nc.vector.tensor_mul(
                ot[:, pair, :], ot[:, pair, :],
                bct[:].rearrange("p a b -> p (a b)"),
            )

    # ---- phase 3: o_proj ----
    with tc.tile_pool(name="ypool", bufs=3) as y_pool, \
         tc.tile_pool(name="psO", bufs=4, space="PSUM") as psO:
        for tc_ in range(16):
            ys = y_pool.tile([128, 1024], BF16, tag="ys")
            for ec in range(2):
                ps = psO.tile([128, 512], F32, tag="acc")
                for pair in range(NPAIR):
                    nc.tensor.matmul(
                        ps[:], ot[:, pair, tc_ * 128:(tc_ + 1) * 128],
                        wo_t[:, pair, ec * 512:(ec + 1) * 512],
                        start=(pair == 0), stop=(pair == NPAIR - 1),
                    )
                nc.vector.tensor_add(
                    ys[:, ec * 512:(ec + 1) * 512], ps[:],
                    bo_t[:, ec * 512:(ec + 1) * 512],
                )
            eng = nc.sync if tc_ % 2 == 0 else nc.scalar
            eng.dma_start(
                out=y[tc_ * 128:(tc_ + 1) * 128, :], in_=ys[:],
            )


def _build(nrep: int = 1, unroll: int = 8):
    """nrep=1: single-shot kernel (used by kernel()). nrep>1: benchmark
    build -- a device-side For_i loop of nrep/unroll iterations, each
    containing `unroll` unrolled copies of the kernel body (dilutes the
    per-iteration all-engine-barrier cost)."""
    key = ("nc", nrep, unroll)
    if key in _CACHE:
        return _CACHE[key]
    from contextlib import ExitStack

    nc = bacc.Bacc("TRN2", target_bir_lowering=False, debug=False, num_devices=8)
    xT = nc.dram_tensor("xT", [D, T], BF16, kind="ExternalInput").ap()
    wqkvT = nc.dram_tensor("wqkvT", [D, 3 * 512], BF16, kind="ExternalInput").ap()
    qkb = nc.dram_tensor("qkb", [128, 8], F32, kind="ExternalInput").ap()
    vb = nc.dram_tensor("vb", [512], F32, kind="ExternalInput").ap()
    woT = nc.dram_tensor("woT", [512, D], BF16, kind="ExternalInput").ap()
    bo = nc.dram_tensor("bo", [D], F32, kind="ExternalInput").ap()
    y = nc.dram_tensor("y", [T, D], BF16, kind="ExternalOutput").ap()

    with tile.TileContext(nc) as tc:
        if nrep == 1:
            with ExitStack() as ctx:
                _emit(nc, tc, ctx, (xT, wqkvT, qkb, vb, woT, bo), (y,))
        else:
            assert nrep % unroll == 0
            with tc.For_i(0, nrep // unroll):
                for u in range(unroll):
                    with ExitStack() as ctx:
                        _emit(nc, tc, ctx, (xT, wqkvT, qkb, vb, woT, bo), (y,),
                              uid=u)
    nc.compile()
    _CACHE[key] = nc
    return nc


def _shard_inputs(x, Wqkv, bqkv, Wo, bo):
    """Build the 8 per-core input maps (x/Wqkv/Wo cast to bf16)."""
    import ml_dtypes
    bf16 = ml_dtypes.bfloat16
    x = np.ascontiguousarray(np.asarray(x, dtype=np.float32))
    Wqkv = np.asarray(Wqkv, dtype=np.float32)
    bqkv = np.asarray(bqkv, dtype=np.float32)
    Wo = np.asarray(Wo, dtype=np.float32)
    bo = np.asarray(bo, dtype=np.float32)

    in_maps = []
    for core in range(8):
        b, hg = core // 2, core % 2
        heads = hg * 8 + np.arange(8)
        rows = (heads[:, None] * 64 + np.arange(64)[None, :]).ravel()  # 512
        q_rows, k_rows, v_rows = rows, 1024 + rows, 2048 + rows
        in_maps.append({
            "xT": np.ascontiguousarray(x[b].T.astype(bf16)),
            "wqkvT": np.ascontiguousarray(
                Wqkv[np.concatenate([q_rows, k_rows, v_rows])].T.astype(bf16)),
            "qkb": np.ascontiguousarray(
                bqkv[np.concatenate([q_rows, k_rows])].reshape(8, 128).T),
            "vb": np.ascontiguousarray(bqkv[v_rows]),
            "woT": np.ascontiguousarray(Wo[:, rows].T.astype(bf16)),
            "bo": (bo if hg == 0 else np.zeros_like(bo)),
        })
    return in_maps


def _get_runner():
    """Build (once) a cached jitted 8-core runner mirroring
    bass2jax.run_bass_via_pjrt, so repeat calls skip retracing."""
    if "runner" in _CACHE:
        return _CACHE["runner"]
    import jax
    from jax.sharding import Mesh, PartitionSpec, NamedSharding
    from jax.experimental.shard_map import shard_map
    from concourse import bass2jax as b2j
    from concourse import mybir as _mb

    nc = _build()
    b2j.install_neuronx_cc_hook()
    partition_name = nc.partition_id_tensor.name if nc.partition_id_tensor else None

    in_names, out_names, out_avals, zero_shapes = [], [], [], []
    for alloc in nc.m.functions[0].allocations:
        if not isinstance(alloc, _mb.MemoryLocationSet):
            continue
        name = alloc.memorylocations[0].name
        if alloc.kind == "ExternalInput":
            if name != partition_name:
                in_names.append(name)
        elif alloc.kind == "ExternalOutput":
            shape = tuple(alloc.tensor_shape)
            dtype = _mb.dt.np(alloc.dtype)
            out_names.append(name)
            out_avals.append(jax.core.ShapedArray(shape, dtype))
            zero_shapes.append((shape, dtype))
    n_params = len(in_names)
    all_names = list(in_names) + list(out_names)
    if partition_name is not None:
        all_names.append(partition_name)

    def _body(*args):
        operands = list(args)
        if partition_name is not None:
            operands.append(b2j.partition_id_tensor())
        outs = b2j._bass_exec_p.bind(
            *operands,
            out_avals=tuple(out_avals),
            in_names=tuple(all_names),
            out_names=tuple(out_names),
            lowering_input_output_aliases=(),
            sim_require_finite=True,
            sim_require_nnan=True,
            nc=nc,
        )
        return tuple(outs)

    devices = jax.devices()[:8]
    mesh = Mesh(np.asarray(devices), ("core",))
    n_outs = len(out_names)
    sharded = jax.jit(
        shard_map(
            _body, mesh=mesh,
            in_specs=(PartitionSpec("core"),) * (n_params + n_outs),
            out_specs=(PartitionSpec("core"),) * n_outs,
            check_rep=False,
        ),
        donate_argnums=tuple(range(n_params, n_params + n_outs)),
        keep_unused=True,
    )
    runner = {
        "sharded": sharded,
        "in_names": in_names,
        "out_names": out_names,
        "zero_shapes": zero_shapes,
        "out_avals": out_avals,
        "shspec": NamedSharding(mesh, PartitionSpec("core")),
    }
    _CACHE["runner"] = runner
    return runner


def _concat_inputs(in_maps, runner):
    return [
        np.concatenate([in_maps[c][name] for c in range(8)], axis=0)
        for name in runner["in_names"]
    ]


def _fresh_zeros(runner):
    return [np.zeros((8 * s[0], *s[1:]), d) for (s, d) in runner["zero_shapes"]]


def kernel(x, Wqkv, bqkv, Wo, bo):
    runner = _get_runner()
    in_maps = _shard_inputs(x, Wqkv, bqkv, Wo, bo)
    out_arrs = runner["sharded"](*_concat_inputs(in_maps, runner),
                                 *_fresh_zeros(runner))
    yi = runner["out_names"].index("y")
    parts = np.asarray(out_arrs[yi]).astype(np.float32).reshape(8, T, D)
    out = np.empty((4, T, D), dtype=np.float32)
    for b in range(4):
        out[b] = parts[2 * b] + parts[2 * b + 1]
    return out
